# revision 18
# baseline (speedup 1.0000x reference)
"""Trainium2 Bass kernel for nn_AttentionModule_27565100105751 (sparse attention).

Sharding: 8 cores = 4 batches x 2 interleaved query-halves.
Core c: batch b = c//2, owns query rows s with s % 2 == h (h = c%2).
Row order on a core: r in [0,1024), global s = 2*r + h; strips of 128 rows
(strip j covers rows r in [128j, 128j+128), causal width T_c = 256(j+1)).

All h-dependence lives in host-prepared data (gathered query tensors and an
additive causal-bias tile), so the single SPMD program is identical on all
cores.

Device pipeline per core:
  - PE-transpose mamba -> relT fp32 (keys) and mambaq -> mqT fp32 (queries)
  - qrT/krT (fp32 matmuls);  x -> xT bf16, xq -> xqT bf16; qT/kT/v (bf16)
  - per strip: rel scores fp32 (+ causal bias via identity-matmul) ->
    top-k threshold via bisection midpoint walk (tensor_scalar is_ge +
    accum counting) -> additive bias mask bm -> QK bf16 + bm via
    identity-matmul -> exp (ACT, constant max bound, accum sumexp) ->
    PE-transpose p -> PV -> Wout -> scale by 1/sumexp -> DMA out.
z_att / switch_loss: data-heavy reductions on device, tiny tail on host.
"""
import math
import os
import sys

for _p in ("/opt/trn_rl_repo", "/root/.axon_site/_ro/trn_rl_repo"):
    if os.path.isdir(_p) and _p not in sys.path:
        sys.path.insert(0, _p)

import numpy as np
import ml_dtypes

import concourse.bass as bass
import concourse.mybir as mybir
import concourse.tile as tile
from concourse import bacc, bass_utils
from concourse.masks import make_identity

B, S, D = 4, 2048, 1024
R_DIM, D_ATT, D_P = 128, 64, 64
TOPK_FRAC = 0.125
SPARSE_FROM = 0
P = 128
NQ = S // 2                  # rows per core
NSTRIP = NQ // P             # 8 strips
NCORES = 8
N_ITERS = 19
NEG_BIG = -4e30
EXP_BIAS = -12.0
F32 = mybir.dt.float32
BF16 = mybir.dt.bfloat16

_compiled = {}


def _build(k_eff):
    nc = bacc.Bacc("TRN2", target_bir_lowering=False, debug=False,
                   num_devices=NCORES)
    AF = mybir.ActivationFunctionType
    OP = mybir.AluOpType

    x_in = nc.dram_tensor("x_b", [S, D], F32, kind="ExternalInput")
    xq_in = nc.dram_tensor("xq_b", [NQ, D], F32, kind="ExternalInput")
    mb_in = nc.dram_tensor("mamba_b", [S, D], F32, kind="ExternalInput")
    mbq_in = nc.dram_tensor("mambaq_b", [NQ, D], F32, kind="ExternalInput")
    cb_in = nc.dram_tensor("cb_bf", [NQ, S], BF16, kind="ExternalInput")
    prev_in = nc.dram_tensor("prev_bf", [NQ, S], BF16, kind="ExternalInput")
    zp_in = nc.dram_tensor("zp_b", [S, D_P], F32, kind="ExternalInput")
    wq_in = nc.dram_tensor("wq_bf", [D, D], BF16, kind="ExternalInput")
    wk_in = nc.dram_tensor("wk_bf", [D, D], BF16, kind="ExternalInput")
    wv_in = nc.dram_tensor("wv_bf", [D, D], BF16, kind="ExternalInput")
    wo_in = nc.dram_tensor("wo_bf", [D, D], BF16, kind="ExternalInput")
    wqr_in = nc.dram_tensor("wqr", [D, R_DIM], F32, kind="ExternalInput")
    wkr_in = nc.dram_tensor("wkr", [D, R_DIM], F32, kind="ExternalInput")
    consts_in = nc.dram_tensor("consts", [P, 4], F32, kind="ExternalInput")

    out_rows = nc.dram_tensor("out_rows", [NQ, D], F32, kind="ExternalOutput")
    summary_o = nc.dram_tensor("summary", [P, 8], F32, kind="ExternalOutput")
    zpsum_o = nc.dram_tensor("zp_sum", [D_P, 1], F32, kind="ExternalOutput")
    cntm_o = nc.dram_tensor("cnt_mask", [P, NSTRIP], F32, kind="ExternalOutput")
    cntb_o = nc.dram_tensor("cnt_both", [P, NSTRIP], F32, kind="ExternalOutput")

    ND = D // P  # 8
    with tile.TileContext(nc) as tc:
        with tc.tile_pool(name="persist", bufs=1) as persist:
            ident_f = persist.tile([P, P], F32, tag="ident_f")
            make_identity(nc, ident_f[:])
            ident_b = persist.tile([P, P], BF16, tag="ident_b")
            make_identity(nc, ident_b[:])
            consts = persist.tile([P, 4], F32, tag="consts")
            nc.sync.dma_start(consts[:], consts_in[:])
            kbias = consts[:, 0:1]
            ebias = consts[:, 1:2]
            ones_col = persist.tile([P, 1], F32, tag="ones_col")
            nc.vector.memset(ones_col[:], 1.0)

            qrT = persist.tile([P, NQ], F32, tag="qrT")
            krT = persist.tile([P, S], F32, tag="krT")
            summary_sb = persist.tile([P, ND], F32, tag="summary_sb")
            cntm_sb = persist.tile([P, NSTRIP], F32, tag="cntm_sb")
            cntb_sb = persist.tile([P, NSTRIP], F32, tag="cntb_sb")

            def copy_ps(dst_ap, src_ap, idx):
                if idx % 2 == 0:
                    nc.scalar.copy(dst_ap, src_ap)
                else:
                    nc.vector.tensor_copy(dst_ap, src_ap)

            def transpose_block(src_tile, nt_chunks, dst_tiles, dst_col, psum_pool,
                                ident, tag):
                """Transpose [P, ND*P] src into ND dst tiles at column dst_col."""
                for g in range(ND // 4):
                    pst = psum_pool.tile([P, 512], F32, tag=tag)
                    for dd in range(4):
                        d = g * 4 + dd
                        nc.tensor.transpose(pst[:, dd * P:(dd + 1) * P],
                                            src_tile[:, d * P:(d + 1) * P],
                                            ident[:])
                    for dd in range(4):
                        d = g * 4 + dd
                        copy_ps(dst_tiles[d][:, dst_col:dst_col + P],
                                pst[:, dd * P:(dd + 1) * P], dst_col // P + d)

            # ---------- phase R: relT/mqT (fp32), qrT/krT ----------
            with tc.tile_pool(name="ldR", bufs=2) as ldR, \
                 tc.tile_pool(name="relT", bufs=1) as prelT, \
                 tc.tile_pool(name="wrel", bufs=1) as pwrel, \
                 tc.tile_pool(name="psR", bufs=4, space="PSUM") as psR, \
                 tc.tile_pool(name="psR2", bufs=2, space="PSUM") as psR2, \
                 tc.tile_pool(name="scrR", bufs=1) as scrR:
                warm = psR2.tile([P, 8], F32, tag="rps", name="warm")
                nc.tensor.matmul(warm[:], ident_f[:], ident_f[:, :8],
                                 start=True, stop=True)
                relT = [prelT.tile([P, S], F32, tag=f"relT{d}", name=f"relT{d}")
                        for d in range(ND)]
                mqT = [prelT.tile([P, NQ], F32, tag=f"mqT{d}", name=f"mqT{d}")
                       for d in range(ND)]
                wqr_sb = pwrel.tile([P, R_DIM * ND], F32, tag="wqr_sb")
                wkr_sb = pwrel.tile([P, R_DIM * ND], F32, tag="wkr_sb")
                for d in range(ND):
                    nc.sync.dma_start(wqr_sb[:, d * R_DIM:(d + 1) * R_DIM],
                                      wqr_in[d * P:(d + 1) * P, :])
                    nc.sync.dma_start(wkr_sb[:, d * R_DIM:(d + 1) * R_DIM],
                                      wkr_in[d * P:(d + 1) * P, :])
                for t in range(S // P):
                    mtile = ldR.tile([P, D], F32, tag="mload")
                    nc.sync.dma_start(mtile[:], mb_in[t * P:(t + 1) * P, :])
                    transpose_block(mtile, ND, relT, t * P, psR, ident_f, "trps")
                for t in range(NQ // P):
                    mtile = ldR.tile([P, D], F32, tag="mload")
                    nc.sync.dma_start(mtile[:], mbq_in[t * P:(t + 1) * P, :])
                    transpose_block(mtile, ND, mqT, t * P, psR, ident_f, "trps")
                for d in range(ND):
                    scr = scrR.tile([P, S], BF16, tag="sumscr")
                    nc.vector.tensor_scalar(
                        out=scr[:], in0=relT[d][:], scalar1=1.0, scalar2=None,
                        op0=OP.mult, op1=OP.add,
                        accum_out=summary_sb[:, d:d + 1])
                nc.sync.dma_start(summary_o[:], summary_sb[:])
                for c in range(NQ // 512):
                    pq = psR2.tile([P, 512], F32, tag="rps")
                    for d in range(ND):
                        nc.tensor.matmul(pq[:],
                                         wqr_sb[:, d * R_DIM:(d + 1) * R_DIM],
                                         mqT[d][:, 512 * c:512 * (c + 1)],
                                         start=(d == 0), stop=(d == ND - 1))
                    nc.scalar.copy(qrT[:, 512 * c:512 * (c + 1)], pq[:])
                for c in range(S // 512):
                    pk = psR2.tile([P, 512], F32, tag="rps", name="pk")
                    for d in range(ND):
                        nc.tensor.matmul(pk[:],
                                         wkr_sb[:, d * R_DIM:(d + 1) * R_DIM],
                                         relT[d][:, 512 * c:512 * (c + 1)],
                                         start=(d == 0), stop=(d == ND - 1))
                    nc.scalar.copy(krT[:, 512 * c:512 * (c + 1)], pk[:])

            # ---------- phase X: xT/xqT (bf16), qT/kT/v ----------
            pmain_cm = tc.tile_pool(name="pmain", bufs=1)
            pmain = pmain_cm.__enter__()
            qT = [pmain.tile([P, NQ], BF16, tag=f"qT{m}", name=f"qT{m}")
                  for m in range(ND)]
            kT = [pmain.tile([P, S], BF16, tag=f"kT{m}", name=f"kT{m}")
                  for m in range(ND)]
            vS = [pmain.tile([P, D], BF16, tag=f"v{t}", name=f"v{t}")
                  for t in range(S // P)]
            with tc.tile_pool(name="ldX", bufs=2) as ldX, \
                 tc.tile_pool(name="xT", bufs=1) as pxT, \
                 tc.tile_pool(name="psXt", bufs=4, space="PSUM") as psXt, \
                 tc.tile_pool(name="psX", bufs=3, space="PSUM") as psX:
                xT = [pxT.tile([P, S], BF16, tag=f"xT{d}", name=f"xT{d}")
                      for d in range(ND)]
                xqT = [pxT.tile([P, NQ], BF16, tag=f"xqT{d}", name=f"xqT{d}")
                       for d in range(ND)]
                for t in range(S // P):
                    xtile = ldX.tile([P, D], F32, tag="xload")
                    nc.sync.dma_start(xtile[:], x_in[t * P:(t + 1) * P, :])
                    transpose_block(xtile, ND, xT, t * P, psXt, ident_f, "trpsX")
                for t in range(NQ // P):
                    xtile = ldX.tile([P, D], F32, tag="xload")
                    nc.sync.dma_start(xtile[:], xq_in[t * P:(t + 1) * P, :])
                    transpose_block(xtile, ND, xqT, t * P, psXt, ident_f, "trpsX")
                with tc.tile_pool(name="wq", bufs=1) as pwq:
                    wq_sb = [pwq.tile([P, D], BF16, tag=f"wq{d}", name=f"wqsb{d}")
                             for d in range(ND)]
                    for d in range(ND):
                        nc.sync.dma_start(wq_sb[d][:], wq_in[d * P:(d + 1) * P, :])
                    for m in range(ND):
                        for c in range(NQ // 512):
                            pq = psX.tile([P, 512], F32, tag="mmps")
                            for d in range(ND):
                                nc.tensor.matmul(
                                    pq[:], wq_sb[d][:, m * P:(m + 1) * P],
                                    xqT[d][:, 512 * c:512 * (c + 1)],
                                    start=(d == 0), stop=(d == ND - 1))
                            copy_ps(qT[m][:, 512 * c:512 * (c + 1)], pq[:], m + c)
                with tc.tile_pool(name="wk", bufs=1) as pwk:
                    wk_sb = [pwk.tile([P, D], BF16, tag=f"wk{d}", name=f"wksb{d}")
                             for d in range(ND)]
                    for d in range(ND):
                        nc.sync.dma_start(wk_sb[d][:], wk_in[d * P:(d + 1) * P, :])
                    for m in range(ND):
                        for c in range(S // 512):
                            pk = psX.tile([P, 512], F32, tag="mmps", name="pkx")
                            for d in range(ND):
                                nc.tensor.matmul(
                                    pk[:], wk_sb[d][:, m * P:(m + 1) * P],
                                    xT[d][:, 512 * c:512 * (c + 1)],
                                    start=(d == 0), stop=(d == ND - 1))
                            copy_ps(kT[m][:, 512 * c:512 * (c + 1)], pk[:], m + c)
                with tc.tile_pool(name="wv", bufs=1) as pwv:
                    wv_sb = [pwv.tile([P, D], BF16, tag=f"wv{d}", name=f"wvsb{d}")
                             for d in range(ND)]
                    for d in range(ND):
                        nc.sync.dma_start(wv_sb[d][:], wv_in[d * P:(d + 1) * P, :])
                    for t in range(S // P):
                        for c in range(2):
                            pv = psX.tile([P, 512], F32, tag="mmps", name="pvx")
                            for d in range(ND):
                                nc.tensor.matmul(
                                    pv[:], xT[d][:, t * P:(t + 1) * P],
                                    wv_sb[d][:, 512 * c:512 * (c + 1)],
                                    start=(d == 0), stop=(d == ND - 1))
                            copy_ps(vS[t][:, 512 * c:512 * (c + 1)], pv[:], t + c)

            # ---------- z_purp sum ----------
            with tc.tile_pool(name="ldZ", bufs=2) as ldZ, \
                 tc.tile_pool(name="psZ", bufs=1, space="PSUM") as psZ:
                pz = psZ.tile([D_P, 1], F32, tag="zps")
                for t in range(S // P):
                    zt = ldZ.tile([P, D_P], F32, tag="zload")
                    nc.sync.dma_start(zt[:], zp_in[t * P:(t + 1) * P, :])
                    nc.tensor.matmul(pz[:], zt[:], ones_col[:],
                                     start=(t == 0), stop=(t == S // P - 1))
                zs = ldZ.tile([D_P, 1], F32, tag="zsum")
                nc.scalar.copy(zs[:], pz[:])
                nc.sync.dma_start(zpsum_o[:], zs[:])

            # ---------- strip loop ----------
            with tc.tile_pool(name="wo", bufs=1) as pwo, \
                 tc.tile_pool(name="rel", bufs=2) as prel, \
                 tc.tile_pool(name="scr", bufs=1) as pscr, \
                 tc.tile_pool(name="bmp", bufs=2) as pbm, \
                 tc.tile_pool(name="pp", bufs=2) as ppp, \
                 tc.tile_pool(name="prev", bufs=1) as pprev, \
                 tc.tile_pool(name="cbp", bufs=1) as pcb, \
                 tc.tile_pool(name="attnT", bufs=2) as patt, \
                 tc.tile_pool(name="outT", bufs=2) as pout, \
                 tc.tile_pool(name="fin", bufs=2) as pfin, \
                 tc.tile_pool(name="small", bufs=2) as psm, \
                 tc.tile_pool(name="ps_rel", bufs=1, space="PSUM") as ps_rel, \
                 tc.tile_pool(name="ps_qk", bufs=2, space="PSUM") as ps_qk, \
                 tc.tile_pool(name="ps_tr", bufs=1, space="PSUM") as ps_tr, \
                 tc.tile_pool(name="ps_o", bufs=2, space="PSUM") as ps_o, \
                 tc.tile_pool(name="ps_f", bufs=2, space="PSUM") as ps_f:
                wo_sb = [pwo.tile([P, D], BF16, tag=f"wo{m}", name=f"wo{m}")
                         for m in range(ND)]
                for m in range(ND):
                    nc.sync.dma_start(wo_sb[m][:], wo_in[m * P:(m + 1) * P, :])
                for j in range(NSTRIP):
                    T_c = 256 * (j + 1)
                    ntc = T_c // P
                    ch512 = [(c * 512, min(512, T_c - c * 512))
                             for c in range((T_c + 511) // 512)]
                    cb = pcb.tile([P, S], BF16, tag="cb")
                    nc.sync.dma_start(cb[:, :T_c],
                                      cb_in[j * P:(j + 1) * P, :T_c])
                    rel = prel.tile([P, S], F32, tag="rel")
                    for (c0, cw) in ch512:
                        pr = ps_rel.tile([P, 512], F32, tag="relps")
                        nc.tensor.matmul(pr[:, :cw],
                                         qrT[:, j * P:(j + 1) * P],
                                         krT[:, c0:c0 + cw],
                                         start=True, stop=False)
                        nc.tensor.matmul(pr[:, :cw], ident_b[:],
                                         cb[:, c0:c0 + cw],
                                         start=False, stop=True)
                        nc.scalar.copy(rel[:, c0:c0 + cw], pr[:, :cw])
                    # strip 0 may contain short rows (s < k_eff) whose
                    # threshold must sink below every causal score, so it
                    # keeps the wide [-8, 8] walk; later strips only need to
                    # bracket the k-th order statistic (|score| < 2 at 5
                    # sigma), so a narrower walk converges in fewer probes.
                    theta0, w0, n_it = (0.0, 4.0, N_ITERS) if j == 0 else \
                        (-0.5, 2.0, N_ITERS - 1)
                    theta = psm.tile([P, 1], F32, tag="theta")
                    nc.vector.memset(theta[:], theta0)
                    hi = psm.tile([P, 1], F32, tag="hi")
                    nc.vector.memset(hi[:], 8.0)
                    cnt = psm.tile([P, 1], F32, tag="cnt")
                    sgn = psm.tile([P, 1], F32, tag="sgn")
                    mle = psm.tile([P, 1], mybir.dt.uint32, tag="mle")
                    scr = pscr.tile([P, S], BF16, tag="cntscr")
                    w = w0
                    for it in range(n_it):
                        nc.vector.tensor_scalar(
                            out=scr[:, :T_c], in0=rel[:, :T_c],
                            scalar1=theta[:], scalar2=None,
                            op0=OP.is_ge, op1=OP.add, accum_out=cnt[:])
                        nc.scalar.activation(out=sgn[:], in_=cnt[:],
                                             func=AF.Sign, bias=kbias,
                                             scale=1.0)
                        # hi := theta where cnt <= k (sgn < 0); bisection's
                        # <=-side probes are monotone decreasing, so hi ends
                        # as the tightest upper threshold.
                        nc.vector.tensor_scalar(
                            out=mle[:], in0=sgn[:], scalar1=0.0, scalar2=None,
                            op0=OP.is_lt)
                        nc.vector.copy_predicated(hi[:], mle[:], theta[:])
                        nc.vector.scalar_tensor_tensor(
                            out=theta[:], in0=sgn[:], scalar=w, in1=theta[:],
                            op0=OP.mult, op1=OP.add)
                        w *= 0.5
                    bm = pbm.tile([P, S], BF16, tag="bm")
                    nc.vector.tensor_scalar(
                        out=bm[:, :T_c], in0=rel[:, :T_c], scalar1=hi[:],
                        scalar2=NEG_BIG, op0=OP.is_lt, op1=OP.mult)
                    nc.vector.tensor_scalar(
                        out=scr[:, :T_c], in0=rel[:, :T_c], scalar1=hi[:],
                        scalar2=None, op0=OP.is_ge, op1=OP.add,
                        accum_out=cntm_sb[:, j:j + 1])
                    prv = pprev.tile([P, S], BF16, tag="prev")
                    nc.sync.dma_start(prv[:, :T_c],
                                      prev_in[j * P:(j + 1) * P, :T_c])
                    scr2 = pscr.tile([P, S], BF16, tag="cntscr", name="scr2")
                    nc.vector.scalar_tensor_tensor(
                        out=scr2[:, :T_c], in0=rel[:, :T_c], scalar=hi[:],
                        in1=prv[:, :T_c], op0=OP.is_ge, op1=OP.mult,
                        accum_out=cntb_sb[:, j:j + 1])
                    p_bf = ppp.tile([P, S], BF16, tag="p")
                    se = psm.tile([P, 4], F32, tag="se")
                    for ci, (c0, cw) in enumerate(ch512):
                        pq = ps_qk.tile([P, 512], F32, tag="qkps")
                        for d in range(ND):
                            nc.tensor.matmul(pq[:, :cw],
                                             qT[d][:, j * P:(j + 1) * P],
                                             kT[d][:, c0:c0 + cw],
                                             start=(d == 0), stop=False)
                        nc.tensor.matmul(pq[:, :cw], ident_b[:],
                                         bm[:, c0:c0 + cw],
                                         start=False, stop=True)
                        nc.scalar.activation(out=p_bf[:, c0:c0 + cw],
                                             in_=pq[:, :cw], func=AF.Exp,
                                             bias=ebias,
                                             scale=1.0 / math.sqrt(D),
                                             accum_out=se[:, ci:ci + 1])
                    sumexp = psm.tile([P, 1], F32, tag="sumexp")
                    nc.vector.tensor_scalar(
                        out=se[:, :len(ch512)], in0=se[:, :len(ch512)],
                        scalar1=1.0, scalar2=None, op0=OP.mult, op1=OP.add,
                        accum_out=sumexp[:])
                    rcp = psm.tile([P, 1], F32, tag="rcp")
                    nc.vector.reciprocal(rcp[:], sumexp[:])
                    att = patt.tile([P, S], BF16, tag="attnT")
                    for g in range((ntc + 3) // 4):
                        tcs = list(range(g * 4, min(g * 4 + 4, ntc)))
                        pst = ps_tr.tile([P, 512], BF16, tag="ptr")
                        for ti, t in enumerate(tcs):
                            nc.tensor.transpose(pst[:, ti * P:(ti + 1) * P],
                                                p_bf[:, t * P:(t + 1) * P],
                                                ident_b[:])
                        gw = len(tcs) * P
                        copy_ps(att[:, g * 512:g * 512 + gw], pst[:, :gw], g)
                    po = [ps_o.tile([P, 512], F32, tag="ops", name=f"ops{j}_{i}")
                          for i in range(2)]
                    for m in range(ND):
                        dst = po[m // 4][:, (m % 4) * P:(m % 4 + 1) * P]
                        for t in range(ntc):
                            nc.tensor.matmul(dst,
                                             vS[t][:, m * P:(m + 1) * P],
                                             att[:, t * P:(t + 1) * P],
                                             start=(t == 0),
                                             stop=(t == ntc - 1))
                    oT = pout.tile([P, D], BF16, tag="oT")
                    nc.scalar.copy(oT[:, :512], po[0][:])
                    nc.vector.tensor_copy(oT[:, 512:], po[1][:])
                    fin = pfin.tile([P, D], F32, tag="fin")
                    for n in range(2):
                        pf = ps_f.tile([P, 512], F32, tag="fps")
                        for m in range(ND):
                            nc.tensor.matmul(pf[:], oT[:, m * P:(m + 1) * P],
                                             wo_sb[m][:, n * 512:(n + 1) * 512],
                                             start=(m == 0), stop=(m == ND - 1))
                        nc.vector.tensor_scalar(
                            out=fin[:, n * 512:(n + 1) * 512], in0=pf[:],
                            scalar1=rcp[:], scalar2=None, op0=OP.mult)
                    nc.sync.dma_start(out_rows[j * P:(j + 1) * P, :], fin[:])
                nc.sync.dma_start(cntm_o[:], cntm_sb[:])
                nc.sync.dma_start(cntb_o[:], cntb_sb[:])
            pmain_cm.__exit__(None, None, None)
    nc.compile()
    return nc


def kernel(x, mamba_out, sal, z_purp, z_cap, warmup, prev_mask, step,
           Wq_rel, Wk_rel, Wgain, bgain, Wsal, bsal, Wpurp, bpurp,
           Wq, Wk, Wv, Wout, _trace=False):
    x = np.asarray(x, dtype=np.float32)
    mamba_out = np.asarray(mamba_out, dtype=np.float32)
    sal = np.asarray(sal, dtype=np.float32)
    z_purp = np.asarray(z_purp, dtype=np.float32)
    z_cap = np.asarray(z_cap, dtype=np.float32)
    warmup = np.asarray(warmup, dtype=np.float32)
    prev_np = np.asarray(prev_mask)

    budget = z_cap / (z_cap + 1e-6)
    k_eff = max(1, int(S * TOPK_FRAC * float(np.mean(budget))))

    if k_eff not in _compiled:
        _compiled[k_eff] = _build(k_eff)
    nc = _compiled[k_eff]

    consts = np.zeros((P, 4), dtype=np.float32)
    consts[:, 0] = -(k_eff + 0.5)
    consts[:, 1] = EXP_BIAS

    w_bf = {n: np.ascontiguousarray(np.asarray(w, np.float32).astype(ml_dtypes.bfloat16))
            for n, w in (("wq_bf", Wq), ("wk_bf", Wk), ("wv_bf", Wv),
                         ("wo_bf", Wout))}
    wqr = np.ascontiguousarray(
        np.asarray(Wq_rel, dtype=np.float32) / np.float32(math.sqrt(R_DIM)))
    wkr = np.ascontiguousarray(np.asarray(Wk_rel, dtype=np.float32))

    t_idx = np.arange(S)
    in_maps = []
    row_s = {}
    for c in range(NCORES):
        b, h = c // 2, c % 2
        s_idx = np.arange(NQ) * 2 + h
        row_s[c] = s_idx
        cb = np.where(t_idx[None, :] <= s_idx[:, None],
                      np.float32(0.0), np.float32(-3e38))
        in_maps.append({
            "x_b": np.ascontiguousarray(x[b]),
            "xq_b": np.ascontiguousarray(x[b][s_idx]),
            "mamba_b": np.ascontiguousarray(mamba_out[b]),
            "mambaq_b": np.ascontiguousarray(mamba_out[b][s_idx]),
            "cb_bf": np.ascontiguousarray(cb.astype(ml_dtypes.bfloat16)),
            "prev_bf": np.ascontiguousarray(
                prev_np[b][s_idx].astype(ml_dtypes.bfloat16)),
            "zp_b": np.ascontiguousarray(z_purp[b]),
            "wq_bf": w_bf["wq_bf"], "wk_bf": w_bf["wk_bf"],
            "wv_bf": w_bf["wv_bf"], "wo_bf": w_bf["wo_bf"],
            "wqr": wqr, "wkr": wkr,
            "consts": consts,
        })

    res = bass_utils.run_bass_kernel_spmd(
        nc, in_maps, core_ids=list(range(NCORES)), trace=_trace)
    kernel._last_res = res

    out = np.zeros((B, S, D), dtype=np.float32)
    total_xor = 0.0
    for c in range(NCORES):
        b, h = c // 2, c % 2
        r = res.results[c]
        s_idx = row_s[c]
        out[b, s_idx, :] = r["out_rows"]
        # cnt arrays are [P(i), NSTRIP(j)]; core row index r = j*128 + i
        cm = r["cnt_mask"].T.reshape(-1).astype(np.float64)
        cb_cnt = r["cnt_both"].T.reshape(-1).astype(np.float64)
        prev_rows = prev_np[b][s_idx]
        prev_tot = prev_rows.sum(axis=1).astype(np.float64)
        short = s_idx <= (k_eff - 1)
        long_m = ~short
        total_xor += float(np.sum(cm[long_m] + prev_tot[long_m]
                                  - 2.0 * cb_cnt[long_m]))
        if short.any():
            prev_low = prev_rows[short][:, :k_eff].sum(axis=1).astype(np.float64)
            total_xor += float(np.sum((k_eff - prev_low)
                                      + (prev_tot[short] - prev_low)))
    switch_loss = np.float32(total_xor / (B * S * S))

    w = float(warmup.reshape(-1)[0])
    z_att = np.zeros((B, D_ATT), dtype=np.float32)
    Wgain = np.asarray(Wgain, np.float32)
    bgain = np.asarray(bgain, np.float32)
    Wsal = np.asarray(Wsal, np.float32)
    bsal = np.asarray(bsal, np.float32)
    Wpurp = np.asarray(Wpurp, np.float32)
    bpurp = np.asarray(bpurp, np.float32)
    for b in range(B):
        r = res.results[2 * b]
        summary = (r["summary"].T.reshape(-1) / S).astype(np.float32)
        zp_mean = (r["zp_sum"].reshape(-1) / S).astype(np.float32)
        purpose = zp_mean @ Wpurp + bpurp
        gate = summary @ Wgain + bgain + purpose + sal[b] @ Wsal + bsal
        learned = 1.0 / (1.0 + np.exp(-gate))
        z_att[b] = (1.0 - w) * 1.0 + w * learned
    st = int(np.asarray(step).reshape(-1)[0]) if np.ndim(step) else int(step)
    if st < SPARSE_FROM:
        switch_loss = np.float32(0.0)
    return (out, z_att.astype(np.float32), switch_loss)


# revision 19
# speedup vs baseline: 1.0326x; 1.0326x over previous
"""Trainium2 Bass kernel for nn_AttentionModule_27565100105751 (sparse attention).

Sharding: 8 cores = 4 batches x 2 interleaved query-halves.
Core c: batch b = c//2, owns query rows s with s % 2 == h (h = c%2).
Row order on a core: r in [0,1024), global s = 2*r + h; strips of 128 rows
(strip j covers rows r in [128j, 128j+128), causal width T_c = 256(j+1)).

All h-dependence lives in host-prepared data (gathered query tensors and an
additive causal-bias tile), so the single SPMD program is identical on all
cores.

Device pipeline per core:
  - PE-transpose mamba -> relT fp32 (keys) and mambaq -> mqT fp32 (queries)
  - qrT/krT (fp32 matmuls);  x -> xT bf16, xq -> xqT bf16; qT/kT/v (bf16)
  - per strip: rel scores fp32 (+ causal bias via identity-matmul) ->
    top-k threshold via bisection midpoint walk (tensor_scalar is_ge +
    accum counting) -> additive bias mask bm -> QK bf16 + bm via
    identity-matmul -> exp (ACT, constant max bound, accum sumexp) ->
    PE-transpose p -> PV -> Wout -> scale by 1/sumexp -> DMA out.
z_att / switch_loss: data-heavy reductions on device, tiny tail on host.
"""
import math
import os
import sys

for _p in ("/opt/trn_rl_repo", "/root/.axon_site/_ro/trn_rl_repo"):
    if os.path.isdir(_p) and _p not in sys.path:
        sys.path.insert(0, _p)

import numpy as np
import ml_dtypes

import concourse.bass as bass
import concourse.mybir as mybir
import concourse.tile as tile
from concourse import bacc, bass_utils
from concourse.masks import make_identity

B, S, D = 4, 2048, 1024
R_DIM, D_ATT, D_P = 128, 64, 64
TOPK_FRAC = 0.125
SPARSE_FROM = 0
P = 128
NQ = S // 2                  # rows per core
NSTRIP = NQ // P             # 8 strips
NCORES = 8
N_ITERS = 19
NEG_BIG = -4e30
EXP_BIAS = -12.0
F32 = mybir.dt.float32
BF16 = mybir.dt.bfloat16

_compiled = {}


def _build(k_eff):
    nc = bacc.Bacc("TRN2", target_bir_lowering=False, debug=False,
                   num_devices=NCORES)
    AF = mybir.ActivationFunctionType
    OP = mybir.AluOpType

    x_in = nc.dram_tensor("x_b", [S, D], BF16, kind="ExternalInput")
    xq_in = nc.dram_tensor("xq_b", [NQ, D], BF16, kind="ExternalInput")
    mb_in = nc.dram_tensor("mamba_b", [S, D], F32, kind="ExternalInput")
    mbq_in = nc.dram_tensor("mambaq_b", [NQ, D], F32, kind="ExternalInput")
    cb_in = nc.dram_tensor("cb_bf", [NQ, S], BF16, kind="ExternalInput")
    prev_in = nc.dram_tensor("prev_bf", [NQ, S], BF16, kind="ExternalInput")
    zp_in = nc.dram_tensor("zp_b", [S, D_P], F32, kind="ExternalInput")
    wq_in = nc.dram_tensor("wq_bf", [D, D], BF16, kind="ExternalInput")
    wk_in = nc.dram_tensor("wk_bf", [D, D], BF16, kind="ExternalInput")
    wv_in = nc.dram_tensor("wv_bf", [D, D], BF16, kind="ExternalInput")
    wo_in = nc.dram_tensor("wo_bf", [D, D], BF16, kind="ExternalInput")
    wqr_in = nc.dram_tensor("wqr", [D, R_DIM], F32, kind="ExternalInput")
    wkr_in = nc.dram_tensor("wkr", [D, R_DIM], F32, kind="ExternalInput")
    consts_in = nc.dram_tensor("consts", [P, 4], F32, kind="ExternalInput")

    out_rows = nc.dram_tensor("out_rows", [NQ, D], F32, kind="ExternalOutput")
    summary_o = nc.dram_tensor("summary", [P, 8], F32, kind="ExternalOutput")
    zpsum_o = nc.dram_tensor("zp_sum", [D_P, 1], F32, kind="ExternalOutput")
    cntm_o = nc.dram_tensor("cnt_mask", [P, NSTRIP], F32, kind="ExternalOutput")
    cntb_o = nc.dram_tensor("cnt_both", [P, NSTRIP], F32, kind="ExternalOutput")

    ND = D // P  # 8
    with tile.TileContext(nc) as tc:
        with tc.tile_pool(name="persist", bufs=1) as persist:
            ident_f = persist.tile([P, P], F32, tag="ident_f")
            make_identity(nc, ident_f[:])
            ident_b = persist.tile([P, P], BF16, tag="ident_b")
            make_identity(nc, ident_b[:])
            consts = persist.tile([P, 4], F32, tag="consts")
            nc.sync.dma_start(consts[:], consts_in[:])
            kbias = consts[:, 0:1]
            ebias = consts[:, 1:2]
            ones_col = persist.tile([P, 1], F32, tag="ones_col")
            nc.vector.memset(ones_col[:], 1.0)

            qrT = persist.tile([P, NQ], F32, tag="qrT")
            krT = persist.tile([P, S], F32, tag="krT")
            summary_sb = persist.tile([P, ND], F32, tag="summary_sb")
            cntm_sb = persist.tile([P, NSTRIP], F32, tag="cntm_sb")
            cntb_sb = persist.tile([P, NSTRIP], F32, tag="cntb_sb")

            def copy_ps(dst_ap, src_ap, idx):
                if idx % 2 == 0:
                    nc.scalar.copy(dst_ap, src_ap)
                else:
                    nc.vector.tensor_copy(dst_ap, src_ap)

            def transpose_block(src_tile, nt_chunks, dst_tiles, dst_col, psum_pool,
                                ident, tag, dtype=F32):
                """Transpose [P, ND*P] src into ND dst tiles at column dst_col."""
                for g in range(ND // 4):
                    pst = psum_pool.tile([P, 512], dtype, tag=tag)
                    for dd in range(4):
                        d = g * 4 + dd
                        nc.tensor.transpose(pst[:, dd * P:(dd + 1) * P],
                                            src_tile[:, d * P:(d + 1) * P],
                                            ident[:])
                    for dd in range(4):
                        d = g * 4 + dd
                        copy_ps(dst_tiles[d][:, dst_col:dst_col + P],
                                pst[:, dd * P:(dd + 1) * P], dst_col // P + d)

            # ---------- phase R: relT/mqT (fp32), qrT/krT ----------
            with tc.tile_pool(name="ldR", bufs=2) as ldR, \
                 tc.tile_pool(name="relT", bufs=1) as prelT, \
                 tc.tile_pool(name="wrel", bufs=1) as pwrel, \
                 tc.tile_pool(name="psR", bufs=4, space="PSUM") as psR, \
                 tc.tile_pool(name="psR2", bufs=2, space="PSUM") as psR2, \
                 tc.tile_pool(name="scrR", bufs=1) as scrR:
                warm = psR2.tile([P, 8], F32, tag="rps", name="warm")
                nc.tensor.matmul(warm[:], ident_f[:], ident_f[:, :8],
                                 start=True, stop=True)
                relT = [prelT.tile([P, S], F32, tag=f"relT{d}", name=f"relT{d}")
                        for d in range(ND)]
                mqT = [prelT.tile([P, NQ], F32, tag=f"mqT{d}", name=f"mqT{d}")
                       for d in range(ND)]
                wqr_sb = pwrel.tile([P, R_DIM * ND], F32, tag="wqr_sb")
                wkr_sb = pwrel.tile([P, R_DIM * ND], F32, tag="wkr_sb")
                for d in range(ND):
                    nc.sync.dma_start(wqr_sb[:, d * R_DIM:(d + 1) * R_DIM],
                                      wqr_in[d * P:(d + 1) * P, :])
                    nc.sync.dma_start(wkr_sb[:, d * R_DIM:(d + 1) * R_DIM],
                                      wkr_in[d * P:(d + 1) * P, :])
                for t in range(S // P):
                    mtile = ldR.tile([P, D], F32, tag="mload")
                    nc.sync.dma_start(mtile[:], mb_in[t * P:(t + 1) * P, :])
                    transpose_block(mtile, ND, relT, t * P, psR, ident_f, "trps")
                for t in range(NQ // P):
                    mtile = ldR.tile([P, D], F32, tag="mload")
                    nc.sync.dma_start(mtile[:], mbq_in[t * P:(t + 1) * P, :])
                    transpose_block(mtile, ND, mqT, t * P, psR, ident_f, "trps")
                for d in range(ND):
                    scr = scrR.tile([P, S], BF16, tag="sumscr")
                    nc.vector.tensor_scalar(
                        out=scr[:], in0=relT[d][:], scalar1=1.0, scalar2=None,
                        op0=OP.mult, op1=OP.add,
                        accum_out=summary_sb[:, d:d + 1])
                nc.sync.dma_start(summary_o[:], summary_sb[:])
                for c in range(NQ // 512):
                    pq = psR2.tile([P, 512], F32, tag="rps")
                    for d in range(ND):
                        nc.tensor.matmul(pq[:],
                                         wqr_sb[:, d * R_DIM:(d + 1) * R_DIM],
                                         mqT[d][:, 512 * c:512 * (c + 1)],
                                         start=(d == 0), stop=(d == ND - 1))
                    nc.scalar.copy(qrT[:, 512 * c:512 * (c + 1)], pq[:])
                for c in range(S // 512):
                    pk = psR2.tile([P, 512], F32, tag="rps", name="pk")
                    for d in range(ND):
                        nc.tensor.matmul(pk[:],
                                         wkr_sb[:, d * R_DIM:(d + 1) * R_DIM],
                                         relT[d][:, 512 * c:512 * (c + 1)],
                                         start=(d == 0), stop=(d == ND - 1))
                    nc.scalar.copy(krT[:, 512 * c:512 * (c + 1)], pk[:])

            # ---------- phase X: xT/xqT (bf16), qT/kT/v ----------
            pmain_cm = tc.tile_pool(name="pmain", bufs=1)
            pmain = pmain_cm.__enter__()
            qT = [pmain.tile([P, NQ], BF16, tag=f"qT{m}", name=f"qT{m}")
                  for m in range(ND)]
            kT = [pmain.tile([P, S], BF16, tag=f"kT{m}", name=f"kT{m}")
                  for m in range(ND)]
            vS = [pmain.tile([P, D], BF16, tag=f"v{t}", name=f"v{t}")
                  for t in range(S // P)]
            with tc.tile_pool(name="ldX", bufs=2) as ldX, \
                 tc.tile_pool(name="xT", bufs=1) as pxT, \
                 tc.tile_pool(name="psXt", bufs=4, space="PSUM") as psXt, \
                 tc.tile_pool(name="psX", bufs=3, space="PSUM") as psX:
                xT = [pxT.tile([P, S], BF16, tag=f"xT{d}", name=f"xT{d}")
                      for d in range(ND)]
                xqT = [pxT.tile([P, NQ], BF16, tag=f"xqT{d}", name=f"xqT{d}")
                       for d in range(ND)]
                for t in range(S // P):
                    xtile = ldX.tile([P, D], BF16, tag="xload")
                    nc.sync.dma_start(xtile[:], x_in[t * P:(t + 1) * P, :])
                    transpose_block(xtile, ND, xT, t * P, psXt, ident_b,
                                    "trpsX", dtype=BF16)
                for t in range(NQ // P):
                    xtile = ldX.tile([P, D], BF16, tag="xload")
                    nc.sync.dma_start(xtile[:], xq_in[t * P:(t + 1) * P, :])
                    transpose_block(xtile, ND, xqT, t * P, psXt, ident_b,
                                    "trpsX", dtype=BF16)
                with tc.tile_pool(name="wq", bufs=1) as pwq:
                    wq_sb = [pwq.tile([P, D], BF16, tag=f"wq{d}", name=f"wqsb{d}")
                             for d in range(ND)]
                    for d in range(ND):
                        nc.sync.dma_start(wq_sb[d][:], wq_in[d * P:(d + 1) * P, :])
                    for m in range(ND):
                        for c in range(NQ // 512):
                            pq = psX.tile([P, 512], F32, tag="mmps")
                            for d in range(ND):
                                nc.tensor.matmul(
                                    pq[:], wq_sb[d][:, m * P:(m + 1) * P],
                                    xqT[d][:, 512 * c:512 * (c + 1)],
                                    start=(d == 0), stop=(d == ND - 1))
                            copy_ps(qT[m][:, 512 * c:512 * (c + 1)], pq[:], m + c)
                with tc.tile_pool(name="wk", bufs=1) as pwk:
                    wk_sb = [pwk.tile([P, D], BF16, tag=f"wk{d}", name=f"wksb{d}")
                             for d in range(ND)]
                    for d in range(ND):
                        nc.sync.dma_start(wk_sb[d][:], wk_in[d * P:(d + 1) * P, :])
                    for m in range(ND):
                        for c in range(S // 512):
                            pk = psX.tile([P, 512], F32, tag="mmps", name="pkx")
                            for d in range(ND):
                                nc.tensor.matmul(
                                    pk[:], wk_sb[d][:, m * P:(m + 1) * P],
                                    xT[d][:, 512 * c:512 * (c + 1)],
                                    start=(d == 0), stop=(d == ND - 1))
                            copy_ps(kT[m][:, 512 * c:512 * (c + 1)], pk[:], m + c)
                with tc.tile_pool(name="wv", bufs=1) as pwv:
                    wv_sb = [pwv.tile([P, D], BF16, tag=f"wv{d}", name=f"wvsb{d}")
                             for d in range(ND)]
                    for d in range(ND):
                        nc.sync.dma_start(wv_sb[d][:], wv_in[d * P:(d + 1) * P, :])
                    for t in range(S // P):
                        for c in range(2):
                            pv = psX.tile([P, 512], F32, tag="mmps", name="pvx")
                            for d in range(ND):
                                nc.tensor.matmul(
                                    pv[:], xT[d][:, t * P:(t + 1) * P],
                                    wv_sb[d][:, 512 * c:512 * (c + 1)],
                                    start=(d == 0), stop=(d == ND - 1))
                            copy_ps(vS[t][:, 512 * c:512 * (c + 1)], pv[:], t + c)

            # ---------- z_purp sum ----------
            with tc.tile_pool(name="ldZ", bufs=2) as ldZ, \
                 tc.tile_pool(name="psZ", bufs=1, space="PSUM") as psZ:
                pz = psZ.tile([D_P, 1], F32, tag="zps")
                for t in range(S // P):
                    zt = ldZ.tile([P, D_P], F32, tag="zload")
                    nc.sync.dma_start(zt[:], zp_in[t * P:(t + 1) * P, :])
                    nc.tensor.matmul(pz[:], zt[:], ones_col[:],
                                     start=(t == 0), stop=(t == S // P - 1))
                zs = ldZ.tile([D_P, 1], F32, tag="zsum")
                nc.scalar.copy(zs[:], pz[:])
                nc.sync.dma_start(zpsum_o[:], zs[:])

            # ---------- strip loop ----------
            with tc.tile_pool(name="wo", bufs=1) as pwo, \
                 tc.tile_pool(name="rel", bufs=2) as prel, \
                 tc.tile_pool(name="scr", bufs=1) as pscr, \
                 tc.tile_pool(name="bmp", bufs=2) as pbm, \
                 tc.tile_pool(name="pp", bufs=2) as ppp, \
                 tc.tile_pool(name="prev", bufs=1) as pprev, \
                 tc.tile_pool(name="cbp", bufs=1) as pcb, \
                 tc.tile_pool(name="attnT", bufs=2) as patt, \
                 tc.tile_pool(name="outT", bufs=2) as pout, \
                 tc.tile_pool(name="fin", bufs=2) as pfin, \
                 tc.tile_pool(name="small", bufs=2) as psm, \
                 tc.tile_pool(name="ps_rel", bufs=1, space="PSUM") as ps_rel, \
                 tc.tile_pool(name="ps_qk", bufs=2, space="PSUM") as ps_qk, \
                 tc.tile_pool(name="ps_tr", bufs=1, space="PSUM") as ps_tr, \
                 tc.tile_pool(name="ps_o", bufs=2, space="PSUM") as ps_o, \
                 tc.tile_pool(name="ps_f", bufs=2, space="PSUM") as ps_f:
                wo_sb = [pwo.tile([P, D], BF16, tag=f"wo{m}", name=f"wo{m}")
                         for m in range(ND)]
                for m in range(ND):
                    nc.sync.dma_start(wo_sb[m][:], wo_in[m * P:(m + 1) * P, :])
                for j in range(NSTRIP):
                    T_c = 256 * (j + 1)
                    ntc = T_c // P
                    ch512 = [(c * 512, min(512, T_c - c * 512))
                             for c in range((T_c + 511) // 512)]
                    cb = pcb.tile([P, S], BF16, tag="cb")
                    nc.sync.dma_start(cb[:, :T_c],
                                      cb_in[j * P:(j + 1) * P, :T_c])
                    rel = prel.tile([P, S], F32, tag="rel")
                    for (c0, cw) in ch512:
                        pr = ps_rel.tile([P, 512], F32, tag="relps")
                        nc.tensor.matmul(pr[:, :cw],
                                         qrT[:, j * P:(j + 1) * P],
                                         krT[:, c0:c0 + cw],
                                         start=True, stop=False)
                        nc.tensor.matmul(pr[:, :cw], ident_b[:],
                                         cb[:, c0:c0 + cw],
                                         start=False, stop=True)
                        nc.scalar.copy(rel[:, c0:c0 + cw], pr[:, :cw])
                    # strip 0 may contain short rows (s < k_eff) whose
                    # threshold must sink below every causal score, so it
                    # keeps the wide [-8, 8] walk; later strips only need to
                    # bracket the k-th order statistic (|score| < 2 at 5
                    # sigma), so a narrower walk converges in fewer probes.
                    theta0, w0, n_it = (0.0, 4.0, N_ITERS) if j == 0 else \
                        (-0.5, 2.0, N_ITERS - 1)
                    theta = psm.tile([P, 1], F32, tag="theta")
                    nc.vector.memset(theta[:], theta0)
                    hi = psm.tile([P, 1], F32, tag="hi")
                    nc.vector.memset(hi[:], 8.0)
                    cnt = psm.tile([P, 1], F32, tag="cnt")
                    sgn = psm.tile([P, 1], F32, tag="sgn")
                    mle = psm.tile([P, 1], mybir.dt.uint32, tag="mle")
                    scr = pscr.tile([P, S], BF16, tag="cntscr")
                    w = w0
                    for it in range(n_it):
                        nc.vector.tensor_scalar(
                            out=scr[:, :T_c], in0=rel[:, :T_c],
                            scalar1=theta[:], scalar2=None,
                            op0=OP.is_ge, op1=OP.add, accum_out=cnt[:])
                        nc.scalar.activation(out=sgn[:], in_=cnt[:],
                                             func=AF.Sign, bias=kbias,
                                             scale=1.0)
                        # hi := theta where cnt <= k (sgn < 0); bisection's
                        # <=-side probes are monotone decreasing, so hi ends
                        # as the tightest upper threshold.
                        nc.vector.tensor_scalar(
                            out=mle[:], in0=sgn[:], scalar1=0.0, scalar2=None,
                            op0=OP.is_lt)
                        nc.vector.copy_predicated(hi[:], mle[:], theta[:])
                        nc.vector.scalar_tensor_tensor(
                            out=theta[:], in0=sgn[:], scalar=w, in1=theta[:],
                            op0=OP.mult, op1=OP.add)
                        w *= 0.5
                    bm = pbm.tile([P, S], BF16, tag="bm")
                    nc.vector.tensor_scalar(
                        out=bm[:, :T_c], in0=rel[:, :T_c], scalar1=hi[:],
                        scalar2=NEG_BIG, op0=OP.is_lt, op1=OP.mult)
                    nc.vector.tensor_scalar(
                        out=scr[:, :T_c], in0=rel[:, :T_c], scalar1=hi[:],
                        scalar2=None, op0=OP.is_ge, op1=OP.add,
                        accum_out=cntm_sb[:, j:j + 1])
                    prv = pprev.tile([P, S], BF16, tag="prev")
                    nc.sync.dma_start(prv[:, :T_c],
                                      prev_in[j * P:(j + 1) * P, :T_c])
                    scr2 = pscr.tile([P, S], BF16, tag="cntscr", name="scr2")
                    nc.vector.scalar_tensor_tensor(
                        out=scr2[:, :T_c], in0=rel[:, :T_c], scalar=hi[:],
                        in1=prv[:, :T_c], op0=OP.is_ge, op1=OP.mult,
                        accum_out=cntb_sb[:, j:j + 1])
                    p_bf = ppp.tile([P, S], BF16, tag="p")
                    se = psm.tile([P, 4], F32, tag="se")
                    for ci, (c0, cw) in enumerate(ch512):
                        pq = ps_qk.tile([P, 512], F32, tag="qkps")
                        for d in range(ND):
                            nc.tensor.matmul(pq[:, :cw],
                                             qT[d][:, j * P:(j + 1) * P],
                                             kT[d][:, c0:c0 + cw],
                                             start=(d == 0), stop=False)
                        nc.tensor.matmul(pq[:, :cw], ident_b[:],
                                         bm[:, c0:c0 + cw],
                                         start=False, stop=True)
                        nc.scalar.activation(out=p_bf[:, c0:c0 + cw],
                                             in_=pq[:, :cw], func=AF.Exp,
                                             bias=ebias,
                                             scale=1.0 / math.sqrt(D),
                                             accum_out=se[:, ci:ci + 1])
                    sumexp = psm.tile([P, 1], F32, tag="sumexp")
                    nc.vector.tensor_scalar(
                        out=se[:, :len(ch512)], in0=se[:, :len(ch512)],
                        scalar1=1.0, scalar2=None, op0=OP.mult, op1=OP.add,
                        accum_out=sumexp[:])
                    rcp = psm.tile([P, 1], F32, tag="rcp")
                    nc.vector.reciprocal(rcp[:], sumexp[:])
                    att = patt.tile([P, S], BF16, tag="attnT")
                    for g in range((ntc + 3) // 4):
                        tcs = list(range(g * 4, min(g * 4 + 4, ntc)))
                        pst = ps_tr.tile([P, 512], BF16, tag="ptr")
                        for ti, t in enumerate(tcs):
                            nc.tensor.transpose(pst[:, ti * P:(ti + 1) * P],
                                                p_bf[:, t * P:(t + 1) * P],
                                                ident_b[:])
                        gw = len(tcs) * P
                        copy_ps(att[:, g * 512:g * 512 + gw], pst[:, :gw], g)
                    po = [ps_o.tile([P, 512], F32, tag="ops", name=f"ops{j}_{i}")
                          for i in range(2)]
                    for m in range(ND):
                        dst = po[m // 4][:, (m % 4) * P:(m % 4 + 1) * P]
                        for t in range(ntc):
                            nc.tensor.matmul(dst,
                                             vS[t][:, m * P:(m + 1) * P],
                                             att[:, t * P:(t + 1) * P],
                                             start=(t == 0),
                                             stop=(t == ntc - 1))
                    oT = pout.tile([P, D], BF16, tag="oT")
                    nc.scalar.copy(oT[:, :512], po[0][:])
                    nc.vector.tensor_copy(oT[:, 512:], po[1][:])
                    fin = pfin.tile([P, D], F32, tag="fin")
                    for n in range(2):
                        pf = ps_f.tile([P, 512], F32, tag="fps")
                        for m in range(ND):
                            nc.tensor.matmul(pf[:], oT[:, m * P:(m + 1) * P],
                                             wo_sb[m][:, n * 512:(n + 1) * 512],
                                             start=(m == 0), stop=(m == ND - 1))
                        nc.vector.tensor_scalar(
                            out=fin[:, n * 512:(n + 1) * 512], in0=pf[:],
                            scalar1=rcp[:], scalar2=None, op0=OP.mult)
                    nc.sync.dma_start(out_rows[j * P:(j + 1) * P, :], fin[:])
                nc.sync.dma_start(cntm_o[:], cntm_sb[:])
                nc.sync.dma_start(cntb_o[:], cntb_sb[:])
            pmain_cm.__exit__(None, None, None)
    nc.compile()
    return nc


def kernel(x, mamba_out, sal, z_purp, z_cap, warmup, prev_mask, step,
           Wq_rel, Wk_rel, Wgain, bgain, Wsal, bsal, Wpurp, bpurp,
           Wq, Wk, Wv, Wout, _trace=False):
    x = np.asarray(x, dtype=np.float32)
    mamba_out = np.asarray(mamba_out, dtype=np.float32)
    sal = np.asarray(sal, dtype=np.float32)
    z_purp = np.asarray(z_purp, dtype=np.float32)
    z_cap = np.asarray(z_cap, dtype=np.float32)
    warmup = np.asarray(warmup, dtype=np.float32)
    prev_np = np.asarray(prev_mask)

    budget = z_cap / (z_cap + 1e-6)
    k_eff = max(1, int(S * TOPK_FRAC * float(np.mean(budget))))

    if k_eff not in _compiled:
        _compiled[k_eff] = _build(k_eff)
    nc = _compiled[k_eff]

    consts = np.zeros((P, 4), dtype=np.float32)
    consts[:, 0] = -(k_eff + 0.5)
    consts[:, 1] = EXP_BIAS

    w_bf = {n: np.ascontiguousarray(np.asarray(w, np.float32).astype(ml_dtypes.bfloat16))
            for n, w in (("wq_bf", Wq), ("wk_bf", Wk), ("wv_bf", Wv),
                         ("wo_bf", Wout))}
    wqr = np.ascontiguousarray(
        np.asarray(Wq_rel, dtype=np.float32) / np.float32(math.sqrt(R_DIM)))
    wkr = np.ascontiguousarray(np.asarray(Wk_rel, dtype=np.float32))

    t_idx = np.arange(S)
    in_maps = []
    row_s = {}
    for c in range(NCORES):
        b, h = c // 2, c % 2
        s_idx = np.arange(NQ) * 2 + h
        row_s[c] = s_idx
        cb = np.where(t_idx[None, :] <= s_idx[:, None],
                      np.float32(0.0), np.float32(-3e38))
        in_maps.append({
            "x_b": np.ascontiguousarray(x[b].astype(ml_dtypes.bfloat16)),
            "xq_b": np.ascontiguousarray(
                x[b][s_idx].astype(ml_dtypes.bfloat16)),
            "mamba_b": np.ascontiguousarray(mamba_out[b]),
            "mambaq_b": np.ascontiguousarray(mamba_out[b][s_idx]),
            "cb_bf": np.ascontiguousarray(cb.astype(ml_dtypes.bfloat16)),
            "prev_bf": np.ascontiguousarray(
                prev_np[b][s_idx].astype(ml_dtypes.bfloat16)),
            "zp_b": np.ascontiguousarray(z_purp[b]),
            "wq_bf": w_bf["wq_bf"], "wk_bf": w_bf["wk_bf"],
            "wv_bf": w_bf["wv_bf"], "wo_bf": w_bf["wo_bf"],
            "wqr": wqr, "wkr": wkr,
            "consts": consts,
        })

    res = bass_utils.run_bass_kernel_spmd(
        nc, in_maps, core_ids=list(range(NCORES)), trace=_trace)
    kernel._last_res = res

    out = np.zeros((B, S, D), dtype=np.float32)
    total_xor = 0.0
    for c in range(NCORES):
        b, h = c // 2, c % 2
        r = res.results[c]
        s_idx = row_s[c]
        out[b, s_idx, :] = r["out_rows"]
        # cnt arrays are [P(i), NSTRIP(j)]; core row index r = j*128 + i
        cm = r["cnt_mask"].T.reshape(-1).astype(np.float64)
        cb_cnt = r["cnt_both"].T.reshape(-1).astype(np.float64)
        prev_rows = prev_np[b][s_idx]
        prev_tot = prev_rows.sum(axis=1).astype(np.float64)
        short = s_idx <= (k_eff - 1)
        long_m = ~short
        total_xor += float(np.sum(cm[long_m] + prev_tot[long_m]
                                  - 2.0 * cb_cnt[long_m]))
        if short.any():
            prev_low = prev_rows[short][:, :k_eff].sum(axis=1).astype(np.float64)
            total_xor += float(np.sum((k_eff - prev_low)
                                      + (prev_tot[short] - prev_low)))
    switch_loss = np.float32(total_xor / (B * S * S))

    w = float(warmup.reshape(-1)[0])
    z_att = np.zeros((B, D_ATT), dtype=np.float32)
    Wgain = np.asarray(Wgain, np.float32)
    bgain = np.asarray(bgain, np.float32)
    Wsal = np.asarray(Wsal, np.float32)
    bsal = np.asarray(bsal, np.float32)
    Wpurp = np.asarray(Wpurp, np.float32)
    bpurp = np.asarray(bpurp, np.float32)
    for b in range(B):
        r = res.results[2 * b]
        summary = (r["summary"].T.reshape(-1) / S).astype(np.float32)
        zp_mean = (r["zp_sum"].reshape(-1) / S).astype(np.float32)
        purpose = zp_mean @ Wpurp + bpurp
        gate = summary @ Wgain + bgain + purpose + sal[b] @ Wsal + bsal
        learned = 1.0 / (1.0 + np.exp(-gate))
        z_att[b] = (1.0 - w) * 1.0 + w * learned
    st = int(np.asarray(step).reshape(-1)[0]) if np.ndim(step) else int(step)
    if st < SPARSE_FROM:
        switch_loss = np.float32(0.0)
    return (out, z_att.astype(np.float32), switch_loss)


# revision 21
# speedup vs baseline: 1.0438x; 1.0108x over previous
"""Trainium2 Bass kernel for nn_AttentionModule_27565100105751 (sparse attention).

Sharding: 8 cores = 4 batches x 2 interleaved query-halves.
Core c: batch b = c//2, owns query rows s with s % 2 == h (h = c%2).
Row order on a core: r in [0,1024), global s = 2*r + h; strips of 128 rows
(strip j covers rows r in [128j, 128j+128), causal width T_c = 256(j+1)).

All h-dependence lives in host-prepared data (gathered query tensors and an
additive causal-bias tile), so the single SPMD program is identical on all
cores.

Device pipeline per core:
  - PE-transpose mamba -> relT fp32 (keys) and mambaq -> mqT fp32 (queries)
  - qrT/krT (fp32 matmuls);  x -> xT bf16, xq -> xqT bf16; qT/kT/v (bf16)
  - per strip: rel scores fp32 (+ causal bias via identity-matmul) ->
    top-k threshold via bisection midpoint walk (tensor_scalar is_ge +
    accum counting) -> additive bias mask bm -> QK bf16 + bm via
    identity-matmul -> exp (ACT, constant max bound, accum sumexp) ->
    PE-transpose p -> PV -> Wout -> scale by 1/sumexp -> DMA out.
z_att / switch_loss: data-heavy reductions on device, tiny tail on host.
"""
import math
import os
import sys

for _p in ("/opt/trn_rl_repo", "/root/.axon_site/_ro/trn_rl_repo"):
    if os.path.isdir(_p) and _p not in sys.path:
        sys.path.insert(0, _p)

import numpy as np
import ml_dtypes

import concourse.bass as bass
import concourse.mybir as mybir
import concourse.tile as tile
from concourse import bacc, bass_utils
from concourse.masks import make_identity

B, S, D = 4, 2048, 1024
R_DIM, D_ATT, D_P = 128, 64, 64
TOPK_FRAC = 0.125
SPARSE_FROM = 0
P = 128
NQ = S // 2                  # rows per core
NSTRIP = NQ // P             # 8 strips
NCORES = 8
N_ITERS = 19
NEG_BIG = -4e30
EXP_BIAS = -12.0
F32 = mybir.dt.float32
BF16 = mybir.dt.bfloat16

_compiled = {}


def _build(k_eff):
    nc = bacc.Bacc("TRN2", target_bir_lowering=False, debug=False,
                   num_devices=NCORES)
    AF = mybir.ActivationFunctionType
    OP = mybir.AluOpType

    x_in = nc.dram_tensor("x_b", [S, D], BF16, kind="ExternalInput")
    xq_in = nc.dram_tensor("xq_b", [NQ, D], BF16, kind="ExternalInput")
    mb_in = nc.dram_tensor("mamba_b", [S, D], F32, kind="ExternalInput")
    mbq_in = nc.dram_tensor("mambaq_b", [NQ, D], F32, kind="ExternalInput")
    cb_in = nc.dram_tensor("cb_bf", [NQ, S], BF16, kind="ExternalInput")
    prev_in = nc.dram_tensor("prev_bf", [NQ, S], BF16, kind="ExternalInput")
    zp_in = nc.dram_tensor("zp_b", [S, D_P], F32, kind="ExternalInput")
    wq_in = nc.dram_tensor("wq_bf", [D, D], BF16, kind="ExternalInput")
    wk_in = nc.dram_tensor("wk_bf", [D, D], BF16, kind="ExternalInput")
    wv_in = nc.dram_tensor("wv_bf", [D, D], BF16, kind="ExternalInput")
    wo_in = nc.dram_tensor("wo_bf", [D, D], BF16, kind="ExternalInput")
    wqr_in = nc.dram_tensor("wqr", [D, R_DIM], F32, kind="ExternalInput")
    wkr_in = nc.dram_tensor("wkr", [D, R_DIM], F32, kind="ExternalInput")
    consts_in = nc.dram_tensor("consts", [P, 4], F32, kind="ExternalInput")

    out_rows = nc.dram_tensor("out_rows", [NQ, D], F32, kind="ExternalOutput")
    summary_o = nc.dram_tensor("summary", [P, 8], F32, kind="ExternalOutput")
    zpsum_o = nc.dram_tensor("zp_sum", [D_P, 1], F32, kind="ExternalOutput")
    cntm_o = nc.dram_tensor("cnt_mask", [P, NSTRIP], F32, kind="ExternalOutput")
    cntb_o = nc.dram_tensor("cnt_both", [P, NSTRIP], F32, kind="ExternalOutput")

    ND = D // P  # 8
    with tile.TileContext(nc) as tc:
        with tc.tile_pool(name="persist", bufs=1) as persist:
            ident_f = persist.tile([P, P], F32, tag="ident_f")
            make_identity(nc, ident_f[:])
            ident_b = persist.tile([P, P], BF16, tag="ident_b")
            make_identity(nc, ident_b[:])
            consts = persist.tile([P, 4], F32, tag="consts")
            nc.sync.dma_start(consts[:], consts_in[:])
            kbias = consts[:, 0:1]
            ebias = consts[:, 1:2]
            ones_col = persist.tile([P, 1], F32, tag="ones_col")
            nc.vector.memset(ones_col[:], 1.0)

            qrT = persist.tile([P, NQ], F32, tag="qrT")
            krT = persist.tile([P, S], F32, tag="krT")
            summary_sb = persist.tile([P, ND], F32, tag="summary_sb")
            cntm_sb = persist.tile([P, NSTRIP], F32, tag="cntm_sb")
            cntb_sb = persist.tile([P, NSTRIP], F32, tag="cntb_sb")

            def copy_ps(dst_ap, src_ap, idx):
                if idx % 2 == 0:
                    nc.scalar.copy(dst_ap, src_ap)
                else:
                    nc.vector.tensor_copy(dst_ap, src_ap)

            def transpose_block(src_tile, nt_chunks, dst_tiles, dst_col, psum_pool,
                                ident, tag, dtype=F32):
                """Transpose [P, ND*P] src into ND dst tiles at column dst_col."""
                for g in range(ND // 4):
                    pst = psum_pool.tile([P, 512], dtype, tag=tag)
                    for dd in range(4):
                        d = g * 4 + dd
                        nc.tensor.transpose(pst[:, dd * P:(dd + 1) * P],
                                            src_tile[:, d * P:(d + 1) * P],
                                            ident[:])
                    for dd in range(4):
                        d = g * 4 + dd
                        copy_ps(dst_tiles[d][:, dst_col:dst_col + P],
                                pst[:, dd * P:(dd + 1) * P], dst_col // P + d)

            # ---------- phase R: relT/mqT (fp32), qrT/krT ----------
            with tc.tile_pool(name="ldR", bufs=2) as ldR, \
                 tc.tile_pool(name="relT", bufs=1) as prelT, \
                 tc.tile_pool(name="wrel", bufs=1) as pwrel, \
                 tc.tile_pool(name="psR", bufs=4, space="PSUM") as psR, \
                 tc.tile_pool(name="psR2", bufs=2, space="PSUM") as psR2, \
                 tc.tile_pool(name="scrR", bufs=1) as scrR:
                warm = psR2.tile([P, 8], F32, tag="rps", name="warm")
                nc.tensor.matmul(warm[:], ident_f[:], ident_f[:, :8],
                                 start=True, stop=True)
                relT = [prelT.tile([P, S], F32, tag=f"relT{d}", name=f"relT{d}")
                        for d in range(ND)]
                mqT = [prelT.tile([P, NQ], F32, tag=f"mqT{d}", name=f"mqT{d}")
                       for d in range(ND)]
                wqr_sb = pwrel.tile([P, R_DIM * ND], F32, tag="wqr_sb")
                wkr_sb = pwrel.tile([P, R_DIM * ND], F32, tag="wkr_sb")
                for d in range(ND):
                    nc.sync.dma_start(wqr_sb[:, d * R_DIM:(d + 1) * R_DIM],
                                      wqr_in[d * P:(d + 1) * P, :])
                    nc.sync.dma_start(wkr_sb[:, d * R_DIM:(d + 1) * R_DIM],
                                      wkr_in[d * P:(d + 1) * P, :])
                for t in range(S // P):
                    mtile = ldR.tile([P, D], F32, tag="mload")
                    nc.sync.dma_start(mtile[:], mb_in[t * P:(t + 1) * P, :])
                    transpose_block(mtile, ND, relT, t * P, psR, ident_f, "trps")
                for t in range(NQ // P):
                    mtile = ldR.tile([P, D], F32, tag="mload")
                    nc.sync.dma_start(mtile[:], mbq_in[t * P:(t + 1) * P, :])
                    transpose_block(mtile, ND, mqT, t * P, psR, ident_f, "trps")
                for d in range(ND):
                    scr = scrR.tile([P, S], BF16, tag="sumscr")
                    nc.vector.tensor_scalar(
                        out=scr[:], in0=relT[d][:], scalar1=1.0, scalar2=None,
                        op0=OP.mult, op1=OP.add,
                        accum_out=summary_sb[:, d:d + 1])
                nc.sync.dma_start(summary_o[:], summary_sb[:])
                for c in range(NQ // 512):
                    pq = psR2.tile([P, 512], F32, tag="rps")
                    for d in range(ND):
                        nc.tensor.matmul(pq[:],
                                         wqr_sb[:, d * R_DIM:(d + 1) * R_DIM],
                                         mqT[d][:, 512 * c:512 * (c + 1)],
                                         start=(d == 0), stop=(d == ND - 1))
                    nc.scalar.copy(qrT[:, 512 * c:512 * (c + 1)], pq[:])
                for c in range(S // 512):
                    pk = psR2.tile([P, 512], F32, tag="rps", name="pk")
                    for d in range(ND):
                        nc.tensor.matmul(pk[:],
                                         wkr_sb[:, d * R_DIM:(d + 1) * R_DIM],
                                         relT[d][:, 512 * c:512 * (c + 1)],
                                         start=(d == 0), stop=(d == ND - 1))
                    nc.scalar.copy(krT[:, 512 * c:512 * (c + 1)], pk[:])

            # ---------- phase X: xT/xqT (bf16), qT/kT/v ----------
            pmain_cm = tc.tile_pool(name="pmain", bufs=1)
            pmain = pmain_cm.__enter__()
            qT = [pmain.tile([P, NQ], BF16, tag=f"qT{m}", name=f"qT{m}")
                  for m in range(ND)]
            kT = [pmain.tile([P, S], BF16, tag=f"kT{m}", name=f"kT{m}")
                  for m in range(ND)]
            vS = [pmain.tile([P, D], BF16, tag=f"v{t}", name=f"v{t}")
                  for t in range(S // P)]
            with tc.tile_pool(name="ldX", bufs=2) as ldX, \
                 tc.tile_pool(name="xT", bufs=1) as pxT, \
                 tc.tile_pool(name="psXt", bufs=4, space="PSUM") as psXt, \
                 tc.tile_pool(name="psX", bufs=3, space="PSUM") as psX:
                xT = [pxT.tile([P, S], BF16, tag=f"xT{d}", name=f"xT{d}")
                      for d in range(ND)]
                xqT = [pxT.tile([P, NQ], BF16, tag=f"xqT{d}", name=f"xqT{d}")
                       for d in range(ND)]
                for t in range(S // P):
                    xtile = ldX.tile([P, D], BF16, tag="xload")
                    nc.sync.dma_start(xtile[:], x_in[t * P:(t + 1) * P, :])
                    transpose_block(xtile, ND, xT, t * P, psXt, ident_b,
                                    "trpsX", dtype=BF16)
                for t in range(NQ // P):
                    xtile = ldX.tile([P, D], BF16, tag="xload")
                    nc.sync.dma_start(xtile[:], xq_in[t * P:(t + 1) * P, :])
                    transpose_block(xtile, ND, xqT, t * P, psXt, ident_b,
                                    "trpsX", dtype=BF16)
                with tc.tile_pool(name="wq", bufs=1) as pwq:
                    wq_sb = [pwq.tile([P, D], BF16, tag=f"wq{d}", name=f"wqsb{d}")
                             for d in range(ND)]
                    for d in range(ND):
                        nc.sync.dma_start(wq_sb[d][:], wq_in[d * P:(d + 1) * P, :])
                    for m in range(ND):
                        for c in range(NQ // 512):
                            pq = psX.tile([P, 512], F32, tag="mmps")
                            for d in range(ND):
                                nc.tensor.matmul(
                                    pq[:], wq_sb[d][:, m * P:(m + 1) * P],
                                    xqT[d][:, 512 * c:512 * (c + 1)],
                                    start=(d == 0), stop=(d == ND - 1))
                            copy_ps(qT[m][:, 512 * c:512 * (c + 1)], pq[:], m + c)
                with tc.tile_pool(name="wk", bufs=1) as pwk:
                    wk_sb = [pwk.tile([P, D], BF16, tag=f"wk{d}", name=f"wksb{d}")
                             for d in range(ND)]
                    for d in range(ND):
                        nc.sync.dma_start(wk_sb[d][:], wk_in[d * P:(d + 1) * P, :])
                    for m in range(ND):
                        for c in range(S // 512):
                            pk = psX.tile([P, 512], F32, tag="mmps", name="pkx")
                            for d in range(ND):
                                nc.tensor.matmul(
                                    pk[:], wk_sb[d][:, m * P:(m + 1) * P],
                                    xT[d][:, 512 * c:512 * (c + 1)],
                                    start=(d == 0), stop=(d == ND - 1))
                            copy_ps(kT[m][:, 512 * c:512 * (c + 1)], pk[:], m + c)
                with tc.tile_pool(name="wv", bufs=1) as pwv:
                    wv_sb = [pwv.tile([P, D], BF16, tag=f"wv{d}", name=f"wvsb{d}")
                             for d in range(ND)]
                    for d in range(ND):
                        nc.sync.dma_start(wv_sb[d][:], wv_in[d * P:(d + 1) * P, :])
                    for t in range(S // P):
                        for c in range(2):
                            pv = psX.tile([P, 512], F32, tag="mmps", name="pvx")
                            for d in range(ND):
                                nc.tensor.matmul(
                                    pv[:], xT[d][:, t * P:(t + 1) * P],
                                    wv_sb[d][:, 512 * c:512 * (c + 1)],
                                    start=(d == 0), stop=(d == ND - 1))
                            copy_ps(vS[t][:, 512 * c:512 * (c + 1)], pv[:], t + c)

            # ---------- z_purp sum ----------
            with tc.tile_pool(name="ldZ", bufs=2) as ldZ, \
                 tc.tile_pool(name="psZ", bufs=1, space="PSUM") as psZ:
                pz = psZ.tile([D_P, 1], F32, tag="zps")
                for t in range(S // P):
                    zt = ldZ.tile([P, D_P], F32, tag="zload")
                    nc.sync.dma_start(zt[:], zp_in[t * P:(t + 1) * P, :])
                    nc.tensor.matmul(pz[:], zt[:], ones_col[:],
                                     start=(t == 0), stop=(t == S // P - 1))
                zs = ldZ.tile([D_P, 1], F32, tag="zsum")
                nc.scalar.copy(zs[:], pz[:])
                nc.sync.dma_start(zpsum_o[:], zs[:])

            # ---------- strip loop ----------
            with tc.tile_pool(name="wo", bufs=1) as pwo, \
                 tc.tile_pool(name="rel", bufs=2) as prel, \
                 tc.tile_pool(name="scr", bufs=1) as pscr, \
                 tc.tile_pool(name="bmp", bufs=2) as pbm, \
                 tc.tile_pool(name="pp", bufs=2) as ppp, \
                 tc.tile_pool(name="prev", bufs=1) as pprev, \
                 tc.tile_pool(name="cbp", bufs=1) as pcb, \
                 tc.tile_pool(name="attnT", bufs=2) as patt, \
                 tc.tile_pool(name="outT", bufs=2) as pout, \
                 tc.tile_pool(name="fin", bufs=2) as pfin, \
                 tc.tile_pool(name="small", bufs=2) as psm, \
                 tc.tile_pool(name="ps_rel", bufs=1, space="PSUM") as ps_rel, \
                 tc.tile_pool(name="ps_qk", bufs=3, space="PSUM") as ps_qk, \
                 tc.tile_pool(name="ps_tr", bufs=1, space="PSUM") as ps_tr, \
                 tc.tile_pool(name="ps_o", bufs=2, space="PSUM") as ps_o, \
                 tc.tile_pool(name="ps_f", bufs=1, space="PSUM") as ps_f:
                wo_sb = [pwo.tile([P, D], BF16, tag=f"wo{m}", name=f"wo{m}")
                         for m in range(ND)]
                for m in range(ND):
                    nc.sync.dma_start(wo_sb[m][:], wo_in[m * P:(m + 1) * P, :])
                for j in range(NSTRIP):
                    T_c = 256 * (j + 1)
                    ntc = T_c // P
                    ch512 = [(c * 512, min(512, T_c - c * 512))
                             for c in range((T_c + 511) // 512)]
                    cb = pcb.tile([P, S], BF16, tag="cb")
                    nc.sync.dma_start(cb[:, :T_c],
                                      cb_in[j * P:(j + 1) * P, :T_c])
                    rel = prel.tile([P, S], F32, tag="rel")
                    for (c0, cw) in ch512:
                        pr = ps_rel.tile([P, 512], F32, tag="relps")
                        nc.tensor.matmul(pr[:, :cw],
                                         qrT[:, j * P:(j + 1) * P],
                                         krT[:, c0:c0 + cw],
                                         start=True, stop=False)
                        nc.tensor.matmul(pr[:, :cw], ident_b[:],
                                         cb[:, c0:c0 + cw],
                                         start=False, stop=True)
                        nc.scalar.copy(rel[:, c0:c0 + cw], pr[:, :cw])
                    # strip 0 may contain short rows (s < k_eff) whose
                    # threshold must sink below every causal score, so it
                    # keeps the wide [-8, 8] walk; later strips only need to
                    # bracket the k-th order statistic (|score| < 2 at 5
                    # sigma), so a narrower walk converges in fewer probes.
                    theta0, w0, n_it = (0.0, 4.0, N_ITERS) if j == 0 else \
                        (-0.5, 2.0, N_ITERS - 1)
                    theta = psm.tile([P, 1], F32, tag="theta")
                    nc.vector.memset(theta[:], theta0)
                    hi = psm.tile([P, 1], F32, tag="hi")
                    nc.vector.memset(hi[:], 8.0)
                    cnt = psm.tile([P, 1], F32, tag="cnt")
                    sgn = psm.tile([P, 1], F32, tag="sgn")
                    mle = psm.tile([P, 1], mybir.dt.uint32, tag="mle")
                    scr = pscr.tile([P, S], BF16, tag="cntscr")
                    w = w0
                    for it in range(n_it):
                        nc.vector.tensor_scalar(
                            out=scr[:, :T_c], in0=rel[:, :T_c],
                            scalar1=theta[:], scalar2=None,
                            op0=OP.is_ge, op1=OP.add, accum_out=cnt[:])
                        nc.scalar.activation(out=sgn[:], in_=cnt[:],
                                             func=AF.Sign, bias=kbias,
                                             scale=1.0)
                        # hi := theta where cnt <= k (sgn < 0); bisection's
                        # <=-side probes are monotone decreasing, so hi ends
                        # as the tightest upper threshold.
                        nc.vector.tensor_scalar(
                            out=mle[:], in0=sgn[:], scalar1=0.0, scalar2=None,
                            op0=OP.is_lt)
                        nc.vector.copy_predicated(hi[:], mle[:], theta[:])
                        nc.vector.scalar_tensor_tensor(
                            out=theta[:], in0=sgn[:], scalar=w, in1=theta[:],
                            op0=OP.mult, op1=OP.add)
                        w *= 0.5
                    bm = pbm.tile([P, S], BF16, tag="bm")
                    nc.vector.tensor_scalar(
                        out=bm[:, :T_c], in0=rel[:, :T_c], scalar1=hi[:],
                        scalar2=NEG_BIG, op0=OP.is_lt, op1=OP.mult)
                    nc.vector.tensor_scalar(
                        out=scr[:, :T_c], in0=rel[:, :T_c], scalar1=hi[:],
                        scalar2=None, op0=OP.is_ge, op1=OP.add,
                        accum_out=cntm_sb[:, j:j + 1])
                    prv = pprev.tile([P, S], BF16, tag="prev")
                    nc.sync.dma_start(prv[:, :T_c],
                                      prev_in[j * P:(j + 1) * P, :T_c])
                    scr2 = pscr.tile([P, S], BF16, tag="cntscr", name="scr2")
                    nc.vector.scalar_tensor_tensor(
                        out=scr2[:, :T_c], in0=rel[:, :T_c], scalar=hi[:],
                        in1=prv[:, :T_c], op0=OP.is_ge, op1=OP.mult,
                        accum_out=cntb_sb[:, j:j + 1])
                    p_bf = ppp.tile([P, S], BF16, tag="p")
                    se = psm.tile([P, 4], F32, tag="se")
                    for ci, (c0, cw) in enumerate(ch512):
                        pq = ps_qk.tile([P, 512], F32, tag="qkps")
                        for d in range(ND):
                            nc.tensor.matmul(pq[:, :cw],
                                             qT[d][:, j * P:(j + 1) * P],
                                             kT[d][:, c0:c0 + cw],
                                             start=(d == 0), stop=False)
                        nc.tensor.matmul(pq[:, :cw], ident_b[:],
                                         bm[:, c0:c0 + cw],
                                         start=False, stop=True)
                        nc.scalar.activation(out=p_bf[:, c0:c0 + cw],
                                             in_=pq[:, :cw], func=AF.Exp,
                                             bias=ebias,
                                             scale=1.0 / math.sqrt(D),
                                             accum_out=se[:, ci:ci + 1])
                    sumexp = psm.tile([P, 1], F32, tag="sumexp")
                    nc.vector.tensor_scalar(
                        out=se[:, :len(ch512)], in0=se[:, :len(ch512)],
                        scalar1=1.0, scalar2=None, op0=OP.mult, op1=OP.add,
                        accum_out=sumexp[:])
                    rcp = psm.tile([P, 1], F32, tag="rcp")
                    nc.vector.reciprocal(rcp[:], sumexp[:])
                    att = patt.tile([P, S], BF16, tag="attnT")
                    for g in range((ntc + 3) // 4):
                        tcs = list(range(g * 4, min(g * 4 + 4, ntc)))
                        pst = ps_tr.tile([P, 512], BF16, tag="ptr")
                        for ti, t in enumerate(tcs):
                            nc.tensor.transpose(pst[:, ti * P:(ti + 1) * P],
                                                p_bf[:, t * P:(t + 1) * P],
                                                ident_b[:])
                        gw = len(tcs) * P
                        copy_ps(att[:, g * 512:g * 512 + gw], pst[:, :gw], g)
                    po = [ps_o.tile([P, 512], F32, tag="ops", name=f"ops{j}_{i}")
                          for i in range(2)]
                    for m in range(ND):
                        dst = po[m // 4][:, (m % 4) * P:(m % 4 + 1) * P]
                        for t in range(ntc):
                            nc.tensor.matmul(dst,
                                             vS[t][:, m * P:(m + 1) * P],
                                             att[:, t * P:(t + 1) * P],
                                             start=(t == 0),
                                             stop=(t == ntc - 1))
                    oT = pout.tile([P, D], BF16, tag="oT")
                    nc.scalar.copy(oT[:, :512], po[0][:])
                    nc.vector.tensor_copy(oT[:, 512:], po[1][:])
                    fin = pfin.tile([P, D], F32, tag="fin")
                    for n in range(2):
                        pf = ps_f.tile([P, 512], F32, tag="fps")
                        for m in range(ND):
                            nc.tensor.matmul(pf[:], oT[:, m * P:(m + 1) * P],
                                             wo_sb[m][:, n * 512:(n + 1) * 512],
                                             start=(m == 0), stop=(m == ND - 1))
                        nc.vector.tensor_scalar(
                            out=fin[:, n * 512:(n + 1) * 512], in0=pf[:],
                            scalar1=rcp[:], scalar2=None, op0=OP.mult)
                    nc.sync.dma_start(out_rows[j * P:(j + 1) * P, :], fin[:])
                nc.sync.dma_start(cntm_o[:], cntm_sb[:])
                nc.sync.dma_start(cntb_o[:], cntb_sb[:])
            pmain_cm.__exit__(None, None, None)
    nc.compile()
    return nc


def kernel(x, mamba_out, sal, z_purp, z_cap, warmup, prev_mask, step,
           Wq_rel, Wk_rel, Wgain, bgain, Wsal, bsal, Wpurp, bpurp,
           Wq, Wk, Wv, Wout, _trace=False):
    x = np.asarray(x, dtype=np.float32)
    mamba_out = np.asarray(mamba_out, dtype=np.float32)
    sal = np.asarray(sal, dtype=np.float32)
    z_purp = np.asarray(z_purp, dtype=np.float32)
    z_cap = np.asarray(z_cap, dtype=np.float32)
    warmup = np.asarray(warmup, dtype=np.float32)
    prev_np = np.asarray(prev_mask)

    budget = z_cap / (z_cap + 1e-6)
    k_eff = max(1, int(S * TOPK_FRAC * float(np.mean(budget))))

    if k_eff not in _compiled:
        _compiled[k_eff] = _build(k_eff)
    nc = _compiled[k_eff]

    consts = np.zeros((P, 4), dtype=np.float32)
    consts[:, 0] = -(k_eff + 0.5)
    consts[:, 1] = EXP_BIAS

    w_bf = {n: np.ascontiguousarray(np.asarray(w, np.float32).astype(ml_dtypes.bfloat16))
            for n, w in (("wq_bf", Wq), ("wk_bf", Wk), ("wv_bf", Wv),
                         ("wo_bf", Wout))}
    wqr = np.ascontiguousarray(
        np.asarray(Wq_rel, dtype=np.float32) / np.float32(math.sqrt(R_DIM)))
    wkr = np.ascontiguousarray(np.asarray(Wk_rel, dtype=np.float32))

    t_idx = np.arange(S)
    in_maps = []
    row_s = {}
    for c in range(NCORES):
        b, h = c // 2, c % 2
        s_idx = np.arange(NQ) * 2 + h
        row_s[c] = s_idx
        cb = np.where(t_idx[None, :] <= s_idx[:, None],
                      np.float32(0.0), np.float32(-3e38))
        in_maps.append({
            "x_b": np.ascontiguousarray(x[b].astype(ml_dtypes.bfloat16)),
            "xq_b": np.ascontiguousarray(
                x[b][s_idx].astype(ml_dtypes.bfloat16)),
            "mamba_b": np.ascontiguousarray(mamba_out[b]),
            "mambaq_b": np.ascontiguousarray(mamba_out[b][s_idx]),
            "cb_bf": np.ascontiguousarray(cb.astype(ml_dtypes.bfloat16)),
            "prev_bf": np.ascontiguousarray(
                prev_np[b][s_idx].astype(ml_dtypes.bfloat16)),
            "zp_b": np.ascontiguousarray(z_purp[b]),
            "wq_bf": w_bf["wq_bf"], "wk_bf": w_bf["wk_bf"],
            "wv_bf": w_bf["wv_bf"], "wo_bf": w_bf["wo_bf"],
            "wqr": wqr, "wkr": wkr,
            "consts": consts,
        })

    res = bass_utils.run_bass_kernel_spmd(
        nc, in_maps, core_ids=list(range(NCORES)), trace=_trace)
    kernel._last_res = res

    out = np.zeros((B, S, D), dtype=np.float32)
    total_xor = 0.0
    for c in range(NCORES):
        b, h = c // 2, c % 2
        r = res.results[c]
        s_idx = row_s[c]
        out[b, s_idx, :] = r["out_rows"]
        # cnt arrays are [P(i), NSTRIP(j)]; core row index r = j*128 + i
        cm = r["cnt_mask"].T.reshape(-1).astype(np.float64)
        cb_cnt = r["cnt_both"].T.reshape(-1).astype(np.float64)
        prev_rows = prev_np[b][s_idx]
        prev_tot = prev_rows.sum(axis=1).astype(np.float64)
        short = s_idx <= (k_eff - 1)
        long_m = ~short
        total_xor += float(np.sum(cm[long_m] + prev_tot[long_m]
                                  - 2.0 * cb_cnt[long_m]))
        if short.any():
            prev_low = prev_rows[short][:, :k_eff].sum(axis=1).astype(np.float64)
            total_xor += float(np.sum((k_eff - prev_low)
                                      + (prev_tot[short] - prev_low)))
    switch_loss = np.float32(total_xor / (B * S * S))

    w = float(warmup.reshape(-1)[0])
    z_att = np.zeros((B, D_ATT), dtype=np.float32)
    Wgain = np.asarray(Wgain, np.float32)
    bgain = np.asarray(bgain, np.float32)
    Wsal = np.asarray(Wsal, np.float32)
    bsal = np.asarray(bsal, np.float32)
    Wpurp = np.asarray(Wpurp, np.float32)
    bpurp = np.asarray(bpurp, np.float32)
    for b in range(B):
        r = res.results[2 * b]
        summary = (r["summary"].T.reshape(-1) / S).astype(np.float32)
        zp_mean = (r["zp_sum"].reshape(-1) / S).astype(np.float32)
        purpose = zp_mean @ Wpurp + bpurp
        gate = summary @ Wgain + bgain + purpose + sal[b] @ Wsal + bsal
        learned = 1.0 / (1.0 + np.exp(-gate))
        z_att[b] = (1.0 - w) * 1.0 + w * learned
    st = int(np.asarray(step).reshape(-1)[0]) if np.ndim(step) else int(step)
    if st < SPARSE_FROM:
        switch_loss = np.float32(0.0)
    return (out, z_att.astype(np.float32), switch_loss)
